# revision 1
# baseline (speedup 1.0000x reference)
"""Trainium2 Bass kernel for nn_AttentionLayer_10591389352529 (sparse window attention).

Reference computation (per batch b, query n):
    local[n,k] = feat gathered at 3x3x3 window around proj_coord[n]   (k=0..26, c=128)
    q[n]       = q_feat[n] @ q_w.T + q_b
    score[n,k] = q[n] . (k_w @ local[n,k] + k_b)
    a[n,:]     = softmax(score[n,:])
    out[n]     = q_feat[n] + sum_k a[n,k] * (v_w @ local[n,k] + v_b)

Algebraic reformulation used here (cuts ~25x the flops):
    score[n,k] = qk[n] . local[n,k] + sb[n]
        where [qk | sb][n] = q_feat[n] @ [q_w.T@k_w | q_w.T@k_b] + [q_b@k_w | q_b.k_b]
    out[n]     = q_feat[n] + v_w @ (sum_k a[n,k] local[n,k]) + v_b     (since sum_k a = 1)

Sharding: flat (B*N)=16384 query axis split across 8 cores (2048 queries each);
each core receives only its batch's feat volume, stored channels-last with the W
axis edge-padded by 1 so each (dd,hh) window row is one contiguous 3*128-float
chunk. The gather runs on-device via indirect DMA with host-precomputed voxel
indices (host does only O(N*9) integer index math + a layout transpose).

Walrus sync-wait limits shape the structure: a Matmult takes only ONE
input-operand wait (all input waits land on the LDWEIGHTS struct) and a DMACopy
takes two. Hence: PE "warmup" matmuls absorb every const-tensor dependency into
PE's vector clock; q_feat (both layouts) is preloaded to SBUF whole; one
indirect gather covers two query tiles (8 gathers over 8 SWDGE lanes -> no
lane-reuse waits); outputs batch through an SBUF staging buffer.
"""

import math
from contextlib import ExitStack

import numpy as np

import concourse.bass as bass
import concourse.tile as tile
from concourse import mybir
from concourse._compat import with_exitstack
from concourse.dve_ops import AFFINE_THEN_ADD, TENSOR_TENSOR_REDUCE
from concourse.tile_rust import add_dep_helper
import concourse.tile_sem_assignment as _tsa

# Single SWDGE completion sem so the kernel-tail Drain's wait list (one per
# touched semaphore) fits the walrus CTRL_NO struct, and so every SWDGE DMA's
# slot-WAW/own-lane/RAW waits merge onto ONE semaphore (one wait slot).
_tsa.NUM_SWDGE_GLOBAL_SEMS = 1

FP32 = mybir.dt.float32
INT32 = mybir.dt.int32
P = 128

B, N, C = 2, 8192, 128
D = H = W = 48
WP = W + 2  # W axis edge-padded by 1 on each side
NVOX = D * H * WP  # rows of the channels-last volume, per batch
NCORES = 8
QPC = B * N // NCORES  # queries per core
KWIN = 27  # 3x3x3 window
KW = KWIN * P  # gathered row length per tile (3456)

# how many of the 27 weighted-sum scaled-copies run on the scalar (ACT) engine;
# the rest run on the vector (DVE) engine. Chosen to balance the two engines'
# per-tile spans (DVE: 27 fused score-reduces + leftovers; ACT: exp + copies).
N_ACT_MAC = 19


@with_exitstack
def attention_body(
    ctx: ExitStack,
    tc: tile.TileContext,
    *,
    featcl: bass.AP,  # (nvox, 128) f32 channels-last padded volume
    qpack: bass.AP,  # (128, 2*qpc + ngath*18) f32 [qft | qf by-tile | gidx bits]
    wpack: bass.AP,  # (128, 257) f32 [q_w.T@k_w | q_w.T@k_b | v_w.T]
    bpack: bass.AP,  # (1, 257) f32   [q_b@k_w | q_b.k_b | v_b]
    out: bass.AP,  # (qpc, 128) f32
    n_act_mac: int = N_ACT_MAC,
):
    nc = tc.nc
    qpc = out.shape[0]
    ntiles = qpc // P
    assert qpc % (2 * P) == 0
    ngath = ntiles // 2
    GBUFS = 4
    # single output DMA at the end: a second DMA writing the same `out` tensor
    # would carry a tensor-level WAW wait on top of its data wait (2 > the
    # 1-wait DMA budget)
    ogroup = ntiles

    const = ctx.enter_context(tc.tile_pool(name="const", bufs=1))
    gath_pool = ctx.enter_context(tc.tile_pool(name="gath", bufs=GBUFS))
    qk_pool = ctx.enter_context(tc.tile_pool(name="qk", bufs=3))
    sc_pool = ctx.enter_context(tc.tile_pool(name="sc", bufs=16))
    small_pool = ctx.enter_context(tc.tile_pool(name="small", bufs=16))
    scratch_pool = ctx.enter_context(tc.tile_pool(name="scratch", bufs=4))
    # >= 28 so scaled-tile slots are never reused within a tile: the WAR wait
    # against PE would otherwise exceed the 1-wait-slot instruction limit
    mac_pool = ctx.enter_context(tc.tile_pool(name="mac", bufs=32))
    m_pool = ctx.enter_context(tc.tile_pool(name="m", bufs=4))
    stage_pool = ctx.enter_context(tc.tile_pool(name="stage", bufs=1))
    psum = ctx.enter_context(tc.tile_pool(name="psum", bufs=2, space="PSUM"))

    # constants / weights / whole q_feat in THREE packed HWDGE DMAs (fewer
    # DMAHW semaphores -> shorter kernel-tail drain wait list):
    #   wpack: [wqk (129) | vwt (128)]            (128, 257)
    #   bpack: [bqk (129) | vb (128)]             (1, 257)
    #   qpack: [qft | qf by-tile | gidx bits]     (128, 2*qpc + ngath*18)
    # const loads ride SWDGE (gpsimd) too: no HWDGE semaphore is ever touched
    qpack_sb = const.tile([P, 2 * qpc + ngath * 18], FP32)
    nc.gpsimd.dma_start(qpack_sb[:], qpack[:, :])
    wpack_sb = const.tile([P, 257], FP32)
    nc.gpsimd.dma_start(wpack_sb[:], wpack[:, :])
    bpack_sb = const.tile([1, 257], FP32)
    nc.gpsimd.dma_start(bpack_sb[:], bpack[:, :])
    wqk_sb = wpack_sb[:, 0:129]
    vwt_sb = wpack_sb[:, 129:257]
    bqk_sb = bpack_sb[:, 0:129]
    vb_sb = bpack_sb[:, 129:257]
    qft_full = qpack_sb[:, 0:qpc]
    qf_full = qpack_sb[:, qpc : 2 * qpc]
    gidx_sb = qpack_sb[:, 2 * qpc :].bitcast(INT32)

    ones_sb = const.tile([1, P], FP32)
    nc.vector.memset(ones_sb[:], 1.0)
    ident = const.tile([P, P], FP32)
    from concourse.masks import make_identity

    make_identity(nc, ident[:])

    # rotating dummy targets for clock-absorbing "touch" copies: rotation keeps
    # each touch's WAW dep old enough to be already-observed (0 extra waits)
    junk_pool = ctx.enter_context(tc.tile_pool(name="junk", bufs=8))

    # PE warmup: walrus's Matmult lowering supports a single input-side
    # sync-wait slot, so absorb every const-tensor dependency into PE's vector
    # clock up front with 1-column matmuls (each waits on one thing only).
    wu = psum.tile([P, 129], FP32, space="PSUM", tag="qk_ps")
    nc.tensor.matmul(wu[:, 0:1], lhsT=ident[:], rhs=ident[:, 0:1], start=True, stop=True)
    for cst in (wpack_sb[:], bpack_sb[:], qpack_sb[:], ones_sb[:]):
        nc.tensor.matmul(
            wu[0:1, 0:1], lhsT=cst[:, 0:1], rhs=cst[:, 0:1], start=True, stop=True
        )

    stage = None
    sig_hist = []
    for t in range(ntiles):
        ts = bass.ts(t, P)
        g, tt = divmod(t, 2)

        if tt == 0:
            # before reusing a gather slot, absorb the previous users' DVE/ACT
            # ticks into the Pool clock via tiny gpsimd touches, so the
            # indirect DMA itself stays within its 2 wait slots
            touches = []
            if g >= GBUFS:
                # tiny SWDGE DMAs reading the reused slot's end-of-use signal
                # cells; SBUF->DRAM (2 wait slots) since each carries one
                # reader-done wait plus possibly its own-lane FIFO wait. The
                # gather itself then only needs its slot WAW wait.
                sg_prev = sig_hist[g - GBUFS]
                jtd = junk_pool.tile([1, 1], FP32, tag="junk_touch_d")
                touches.append(nc.gpsimd.tensor_copy(jtd[:], sg_prev[0:1, 0:1]))
                jta = junk_pool.tile([1, 1], FP32, tag="junk_touch_a")
                touches.append(nc.gpsimd.tensor_copy(jta[:], sg_prev[0:1, 1:2]))
            # gather for tiles 2g and 2g+1: 18 chunks x (3 vox * 128 ch), one
            # indirect DMA per chunk index j (the HW DGE consumes exactly ONE
            # index per partition and streams the whole dest row from it).
            # Separate tiles per j avoid intra-period WAW serialization.
            # j < 9 -> tile 2g, j >= 9 -> tile 2g+1.
            gath = [
                gath_pool.tile([P, 3 * P], FP32, tag=f"gath{j}", name=f"gath{j}_{g}")
                for j in range(18)
            ]
            for j in range(18):
                gi = nc.gpsimd.indirect_dma_start(
                    out=gath[j][:],
                    out_offset=None,
                    in_=featcl[:, :],
                    in_offset=bass.IndirectOffsetOnAxis(
                        ap=gidx_sb[:, g * 18 + j : g * 18 + j + 1], axis=0
                    ),
                )
                # keep the scheduler from hoisting the gather above the
                # touches that pre-absorb its WAR ticks
                for tch in touches:
                    add_dep_helper(
                        gi.ins, tch.ins, sync=False, reason="gather after touches"
                    )

        def slab(k):
            j, v = tt * 9 + k // 3, k % 3
            return gath[j][:, v * P : (v + 1) * P]

        # ---- qk = q_feat @ [W1|w2] + [c1|c2]  (PE) ----
        qk_ps = psum.tile([P, 129], FP32, space="PSUM", tag="qk_ps")
        nc.tensor.matmul(
            qk_ps[:], lhsT=qft_full[:, ts], rhs=wqk_sb[:], start=True, stop=False
        )
        nc.tensor.matmul(
            qk_ps[:], lhsT=ones_sb[:], rhs=bqk_sb[:], start=False, stop=True
        )
        # ACT is nearer PSUM; the cross-engine wait this puts on the first
        # score op is legalized into an EventSemaphore by _legalize_waits.
        qk_sb = qk_pool.tile([P, 129], FP32)
        nc.scalar.copy(qk_sb[:], qk_ps[:])

        # ---- scores[n,k] = qk[n] . slab_k[n] + sb[n]  (DVE, fused custom op:
        #      accum_out = s0 + sum(in0*in1*s1); the plain TENSOR_TENSOR_REDUCE
        #      ISA opcode crashes this runtime) ----
        scores = sc_pool.tile([P, KWIN], FP32, tag="scores")
        for k in range(KWIN):
            scr = scratch_pool.tile([P, P], FP32, tag="ttr_scr")
            nc.vector._custom_dve(
                TENSOR_TENSOR_REDUCE,
                out=scr[:],
                in0=qk_sb[:, 0:P],
                in1=slab(k),
                s0=qk_sb[:, P : P + 1],
                s1=1.0,
                accum_out=scores[:, k : k + 1],
            )

        # ---- softmax pieces (max-subtraction required: the ACT exp spline
        # returns non-finite values outside its fitted range on HW) ----
        negmax = small_pool.tile([P, 1], FP32, tag="negmax")
        nc.vector.tensor_reduce(
            out=negmax[:],
            in_=scores[:],
            axis=mybir.AxisListType.X,
            op=mybir.AluOpType.max,
            negate=True,
        )
        e = sc_pool.tile([P, KWIN], FP32, tag="e")
        sumexp = small_pool.tile([P, 1], FP32, tag="sumexp")
        nc.scalar.activation(
            e[:],
            scores[:],
            mybir.ActivationFunctionType.Exp,
            bias=negmax[:],
            scale=1.0,
            accum_out=sumexp[:],
        )
        recip = small_pool.tile([P, 1], FP32, tag="recip")
        nc.vector.reciprocal(recip[:], sumexp[:])
        # absorb recip (DVE) into ACT's clock so the mr_sb scaled-copy below
        # only needs its PE wait
        jr = junk_pool.tile([1, 1], FP32, tag="junk_recip")
        nc.scalar.copy(jr[:], recip[0:1, :])

        # ---- m = sum_k e[:,k] * slab_k, accumulated in PSUM via identity
        #      matmuls; the 1/sumexp normalization is folded into the
        #      PSUM->SBUF copy ----
        m_ps = psum.tile([P, P], FP32, space="PSUM", tag="m_ps")
        for k in range(n_act_mac):
            scaled = mac_pool.tile([P, P], FP32, tag="scaled")
            nc.scalar.mul(scaled[:], slab(k), e[:, k : k + 1])
            last_act_scaled = scaled
            nc.tensor.matmul(
                m_ps[:], lhsT=ident[:], rhs=scaled[:], start=(k == 0), stop=False
            )
        # DVE's share accumulates in SBUF via a fused multiply-add chain
        # (acc = slab*e_k + acc) -> one PE merge matmul instead of 7
        acc = mac_pool.tile([P, P], FP32, tag="dveacc")
        nc.vector.tensor_scalar_mul(acc[:], slab(n_act_mac), e[:, n_act_mac : n_act_mac + 1])
        for k in range(n_act_mac + 1, KWIN):
            acc2 = mac_pool.tile([P, P], FP32, tag="dveacc")
            nc.vector._custom_dve(
                AFFINE_THEN_ADD,
                out=acc2[:],
                in0=slab(k),
                in1=acc[:],
                s0=e[:, k : k + 1],
                s1=0.0,
            )
            acc = acc2
        last_dve_scaled = acc
        nc.tensor.matmul(m_ps[:], lhsT=ident[:], rhs=acc[:], start=False, stop=True)
        mr_sb = m_pool.tile([P, P], FP32, tag="mr_sb")
        nc.scalar.mul(mr_sb[:], m_ps[:], recip[:])

        # ---- x = mr @ v_w.T + q_feat + v_b  (PE; transpose mr first) ----
        mt_ps = psum.tile([P, P], FP32, space="PSUM", tag="mt_ps")
        nc.tensor.transpose(out=mt_ps[:], in_=mr_sb[:], identity=ident[:])
        mt_sb = m_pool.tile([P, P], FP32, tag="mt_sb")
        nc.vector.tensor_copy(mt_sb[:], mt_ps[:])

        x_ps = psum.tile([P, P], FP32, space="PSUM", tag="x_ps")
        nc.tensor.matmul(x_ps[:], lhsT=mt_sb[:], rhs=vwt_sb[:], start=True, stop=False)
        nc.tensor.matmul(
            x_ps[:],
            lhsT=ident[:],
            rhs=qf_full[:, t * P : (t + 1) * P],
            start=False,
            stop=False,
        )
        nc.tensor.matmul(x_ps[:], lhsT=ones_sb[:], rhs=vb_sb[:], start=False, stop=True)

        # ---- stage output; one SWDGE DMA at the end. A gpsimd touch of the
        # staging buffer first absorbs the ACT copies' ticks into the SWDGE
        # stream so the out DMA needs only its own-sem wait ----
        if t % ogroup == 0:
            stage = stage_pool.tile([P, ogroup * P], FP32, tag="stage")
        nc.scalar.copy(stage[:, bass.ts(t % ogroup, P)], x_ps[:])
        if t % ogroup == ogroup - 1:
            t0 = t - (ogroup - 1)
            jout = junk_pool.tile([1, 1], FP32, tag="junk_out")
            otch = nc.gpsimd.tensor_copy(jout[:], stage[0:1, ogroup * P - 1 :])
            od = nc.gpsimd.dma_start(
                out.rearrange("(t p) c -> p t c", p=P)[:, t0 : t0 + ogroup, :],
                stage[:].rearrange("p (t c) -> p t c", c=P),
            )
            add_dep_helper(od.ins, otch.ins, sync=False, reason="out after stage touch")

        if tt == 1:
            # end-of-gather-period signals: each reads the LAST scaled tile
            # its engine produced this tile, so the write is necessarily
            # scheduled after that engine's final `gath` read; a touch of this
            # tile before the slot-reusing gather then implies all prior
            # readers are done
            sg = junk_pool.tile([1, 2], FP32, tag="sig")
            nc.vector.tensor_copy(sg[0:1, 0:1], last_dve_scaled[0:1, 0:1])
            nc.scalar.copy(sg[0:1, 1:2], last_act_scaled[0:1, 0:1])
            sig_hist.append(sg)


def build_program(qpc: int, nvox: int, n_act_mac: int = N_ACT_MAC):
    """Build the SPMD Bass program. Returns nc."""
    nc = bass.Bass("TRN2", target_bir_lowering=False, debug=False, num_devices=NCORES)
    ngath = qpc // (2 * P)
    aps = {}
    decl = [
        ("featcl", (nvox, C), FP32, False),
        ("qpack", (C, 2 * qpc + ngath * 18), FP32, False),
        ("wpack", (C, 257), FP32, False),
        ("bpack", (1, 257), FP32, False),
        ("out", (qpc, C), FP32, True),
    ]
    for name, shape, dt, is_out in decl:
        kind = "ExternalOutput" if is_out else "ExternalInput"
        aps[name] = nc.dram_tensor(name, list(shape), dt, kind=kind).ap()
    with tile.TileContext(nc) as tc:
        attention_body(tc, n_act_mac=n_act_mac, **aps)
    # populate .instr bytes for InstISA subclasses (TensorTensorReduce);
    # Bacc.compile() does this but the raw-Bass path does not.
    mybir.codegen_inst_isa_subclasses(nc)
    _legalize_waits(nc)
    return nc


def _legalize_waits(nc, max_waits: int = 1):
    """This walrus build accepts only ONE sync-wait slot per instruction
    struct. For any instruction Tile scheduled with more waits, keep the last
    and hoist the rest onto preceding same-engine EventSemaphore instructions
    (the engine queue is FIFO, so waiting before the instruction is
    equivalent to waiting on it)."""
    for f in nc.m.functions:
        for blk in f.blocks:
            insts = blk.instructions
            i = 0
            while i < len(insts):
                inst = insts[i]
                si = inst.sync_info
                if si is not None and len(si.on_wait) > max_waits:
                    waits = list(si.on_wait)
                    pre = []
                    while len(waits) > max_waits:
                        chunk, waits = waits[:max_waits], waits[max_waits:]
                        pre.append(
                            mybir.InstEventSemaphore(
                                name=f"{inst.name}-ws{len(pre)}",
                                engine=inst.engine,
                                ins=[],
                                outs=[],
                                bass_nofuse=True,
                                sync_info=mybir.SyncInfo(on_wait=chunk, on_update=[]),
                            )
                        )
                    si.on_wait = waits
                    insts[i:i] = pre
                    i += len(pre)
                i += 1


def pack_queries(qf_c: np.ndarray, gidx_c: np.ndarray) -> np.ndarray:
    """Build the (128, 2*qpc + ngath*18) qpack host tensor:
    [ qft | qf by-tile (p, t, c) | gather chunk indices (p, g, tt*9+j) ]."""
    qpc = qf_c.shape[0]
    ntiles = qpc // P
    ngath = ntiles // 2
    qft = qf_c.T  # (128, qpc)
    qf_bytile = qf_c.reshape(ntiles, P, C).transpose(1, 0, 2).reshape(P, qpc)
    g = gidx_c.reshape(ngath, 2, P, 9)  # (g, tt, p, j)
    g = np.transpose(g, (2, 0, 1, 3)).reshape(P, ngath * 18)  # (p, g*18 + tt*9 + j)
    return np.ascontiguousarray(
        np.concatenate([qft, qf_bytile, g.view(np.float32)], axis=1, dtype=np.float32)
    )


def host_prepare(q_feat, feat, proj_coord, q_w, q_b, k_w, k_b, v_w, v_b):
    """All host-side input marshalling. Returns per-core input maps."""
    q_feat = np.asarray(q_feat, dtype=np.float32)
    feat = np.asarray(feat, dtype=np.float32)
    proj_coord = np.asarray(proj_coord, dtype=np.int32)
    q_w, q_b, k_w, k_b, v_w, v_b = (
        np.asarray(a, dtype=np.float32) for a in (q_w, q_b, k_w, k_b, v_w, v_b)
    )

    # channels-last volume with W edge-padded by 1: (B, D, H, WP, C)
    fcl = np.transpose(feat, (0, 2, 3, 4, 1))  # (B,D,H,W,C)
    fcl = np.pad(fcl, ((0, 0), (0, 0), (0, 0), (1, 1), (0, 0)), mode="edge")
    fcl = np.ascontiguousarray(fcl.reshape(B, NVOX, C))

    # voxel row index of the first (w-1) voxel of each (dd,hh) chunk
    d = proj_coord[..., 0].astype(np.int64)
    h = proj_coord[..., 1].astype(np.int64)
    w = proj_coord[..., 2].astype(np.int64)
    offs = [(dd, hh) for dd in (-1, 0, 1) for hh in (-1, 0, 1)]
    gidx = np.empty((B, N, 9), dtype=np.int32)
    for j, (dd, hh) in enumerate(offs):
        dc = np.clip(d + dd, 0, D - 1)
        hc = np.clip(h + hh, 0, H - 1)
        gidx[..., j] = ((dc * H + hc) * WP + w).astype(np.int32)

    # folded weights
    wqk = np.concatenate([q_w.T @ k_w, (q_w.T @ k_b)[:, None]], axis=1)  # (128,129)
    bqk = np.concatenate([q_b @ k_w, [q_b @ k_b]])[None, :]  # (1,129)
    wpack = np.ascontiguousarray(
        np.concatenate([wqk, v_w.T], axis=1, dtype=np.float32)
    )  # (128,257)
    bpack = np.ascontiguousarray(
        np.concatenate([bqk, v_b[None, :]], axis=1, dtype=np.float32)
    )  # (1,257)

    qf_flat = q_feat.reshape(B * N, C)
    gidx_flat = gidx.reshape(B * N, 9)

    in_maps = []
    for core in range(NCORES):
        lo = core * QPC
        hi = lo + QPC
        b = lo // N  # each core's queries live in a single batch
        assert (hi - 1) // N == b
        qf_c = np.ascontiguousarray(qf_flat[lo:hi])
        in_maps.append(
            {
                "featcl": fcl[b],
                "qpack": pack_queries(qf_c, gidx_flat[lo:hi]),
                "wpack": wpack,
                "bpack": bpack,
            }
        )
    return in_maps


_PROGRAM_CACHE = {}


def _get_program():
    key = (QPC, NVOX, N_ACT_MAC)
    if key not in _PROGRAM_CACHE:
        _PROGRAM_CACHE[key] = build_program(QPC, NVOX)
    return _PROGRAM_CACHE[key]


def run_on_hw(in_maps, trace=False, **kwargs):
    from concourse.bass_utils import run_bass_kernel_spmd

    nc = _get_program()
    return run_bass_kernel_spmd(nc, in_maps, list(range(NCORES)), trace=trace, **kwargs)


def kernel(q_feat, feat, proj_coord, hr_coord=None, q_w=None, q_b=None, k_w=None,
           k_b=None, v_w=None, v_b=None, **_unused):
    """Full inputs in, full output out. hr_coord is unused by the reference."""
    in_maps = host_prepare(q_feat, feat, proj_coord, q_w, q_b, k_w, k_b, v_w, v_b)
    res = run_on_hw(in_maps)
    parts = [res.results[c]["out"] for c in range(NCORES)]
    out = np.concatenate(parts, axis=0).reshape(B, N, C).astype(np.float32)
    return out

